# revision 17
# baseline (speedup 1.0000x reference)
"""Trainium2 Bass kernel for a 16-head self-attention block.

Model (matches the nn.Module reference):
    q = x @ Wq + bq; k = x @ Wk + bk; v = x @ Wv + bv   (per-head split, Hd=64)
    attn = softmax(q k^T / sqrt(Hd)); out = (attn v) @ Wo + bo
Shapes: x [2, 2048, 1024], 16 heads, head dim 64.

Sharding (8 cores): core = (batch b in {0,1}) x (head-group g in {0..3});
each core owns 4 heads of one batch element. Inputs are sliced on the host;
each core returns a partial y^T = (attended_g @ Wo_g)^T which the host sums
over the 4 head-groups per batch.

Per-core design (bf16 matmul operands, fp32 PSUM accumulation — fp32r runs
fp32_mode=HIGH two-pass on hardware, ~2x the bf16 stream time):
  - Host passes xT = x[b]^T so projections need no on-device transpose.
  - Scores are computed transposed, S^T[key, q] = K_h Q_h^T, so softmax's
    exp runs straight out of PSUM on the Scalar engine and A = P V consumes
    P^T with no transpose anywhere. The two heads of a pair run as
    CONCURRENT row-group matmuls (K=64 at PE row offsets 0/64), so a score
    pair costs ~one 512-row pass.
  - softmax skips the max subtraction (mathematically identical; scores are
    O(5) here and ACT exp is <=2 ULP on [-10,10]).
  - P row sums come from a ones column appended to V (A' = P [V|1]), so the
    A.V matmul (M=65) yields attended^T plus the sums row with no extra
    matmuls or PSUM banks.
  - Normalization: sums row -> partition 0 via tiny SBUF->SBUF DMA, fast
    approx reciprocal (+cast to bf16), broadcast to 64 rows with a K=1 ones
    outer-product on the PE, then one vector multiply per head.
  - 1/sqrt(Hd) is folded into Wq (and bq) on the host; bv and bo are folded
    in exactly on the host: y += bo + bv @ Wo (softmax rows sum to 1).

Scheduling notes (exp stream is the critical resource, ~1.15us per key
chunk; everything else must fit underneath it):
  - PSUM banks: scores 2x[128,2,512] (4) + attended 3x[65,512] (3) +
    scratch ring 1x[128,512] (1) = 8. Three attended bufs let the next
    pair's first A.V start before the previous pair's PSUM eviction.
  - V is projected just-in-time during block 0 pair 0, two key chunks per
    PSUM tile.
  - The Q^T halves for blocks 2/3 are emitted during block 1 as
    single-bank groups in the scratch ring (not the score ring, which
    would stall the exp stream behind their deprioritized eviction).
  - Output projection of block qb is emitted a few chunks into block qb+1
    (PE slack). For the LAST block, pair 0's half runs during pair 1's
    c-loop into u_sb, and the tail only runs pair 1's matmuls + an add.
  - Input DMAs are split across the two hardware DGE queues (sync + ACT)
    and wk/wq are sliced per contraction chunk so the first projection
    matmuls start as soon as x chunk 0 lands.
"""

import numpy as np

import concourse.bass as bass
import concourse.tile as tile
from concourse import bacc
from concourse import mybir

P = 128          # partitions
S = 2048         # sequence length
D = 1024         # model dim
H = 16           # total heads
HD = 64          # head dim
G = 4            # heads per core
GD = G * HD      # 256 head-group dims per core
NQB = 4          # query blocks
QB = S // NQB    # 512
NKC = S // P     # 16 key chunks
NDC = D // P     # 8 contraction chunks
F32 = mybir.dt.float32
BF16 = mybir.dt.bfloat16

TRACE = False
LAST_RESULTS = None


def _build_nc(nqb=NQB, do_attn=True, do_exp=True, do_outproj=True,
              do_norm=True, do_proj=True, do_dma=True):
    nc = bacc.Bacc(trn_type="TRN2")
    xT = nc.dram_tensor("xT", [D, S], BF16, kind="ExternalInput")
    wq = nc.dram_tensor("wq", [D, GD], BF16, kind="ExternalInput")
    wk = nc.dram_tensor("wk", [D, GD], BF16, kind="ExternalInput")
    wv = nc.dram_tensor("wv", [D, GD], BF16, kind="ExternalInput")
    wo = nc.dram_tensor("wo", [GD, D], BF16, kind="ExternalInput")
    bias = nc.dram_tensor("bias", [P, 4], F32, kind="ExternalInput")
    cst = nc.dram_tensor("cst", [P, 768], BF16, kind="ExternalInput")
    yT = nc.dram_tensor("yT", [D, S], BF16, kind="ExternalOutput")

    Exp = mybir.ActivationFunctionType.Exp
    Ident = mybir.ActivationFunctionType.Identity

    with tile.TileContext(nc) as tc, \
         tc.tile_pool(name="sb", bufs=1) as sb, \
         tc.tile_pool(name="pt", bufs=3) as ptp, \
         tc.tile_pool(name="small", bufs=2) as smp, \
         tc.tile_pool(name="attnp", bufs=5) as atp, \
         tc.tile_pool(name="tiny", bufs=2) as tnp, \
         tc.tile_pool(name="ps_s", bufs=2, space="PSUM") as ps_s, \
         tc.tile_pool(name="ps_av", bufs=3, space="PSUM") as ps_av, \
         tc.tile_pool(name="ps_y", bufs=1, space="PSUM") as ps_y:

        # ---- persistent SBUF tensors
        wq_sb = sb.tile([P, NDC, GD], BF16, tag="wq")
        wk_sb = sb.tile([P, NDC, GD], BF16, tag="wk")
        wv_sb = sb.tile([P, NDC, GD], BF16, tag="wv")
        wo_sb = sb.tile([P, 2, D], BF16, tag="wo")   # [pair-dims, pair, out-dim]
        bias_sb = sb.tile([P, 4], F32, tag="bias")
        scratch = sb.tile([P, 1], F32, tag="scratch")
        cst_sb = sb.tile([P, 768], BF16, tag="cst")
        ones_col = cst_sb[:, 0:1]             # [128, 1] ones
        ones_row = cst_sb[0:1, 641:641 + HD]  # [1, 64] ones
        ones8 = cst_sb[:, 740:748]            # [128, 8] ones
        u_sb = sb.tile([P, NDC, QB], F32, tag="u")   # last-block pair-0 y half
        x_sb = [sb.tile([P, S], BF16, tag=f"x{d}", name=f"x{d}") for d in range(NDC)]
        kT = [sb.tile([P, S], BF16, tag=f"k{p}", name=f"k{p}") for p in range(2)]
        qT = [sb.tile([P, S], BF16, tag=f"q{p}", name=f"q{p}") for p in range(2)]
        # V with a ones column per head, two key chunks per tile:
        # [keys, chunk-pair half, head, 65]
        v_sb = [sb.tile([P, 2, G, HD + 1], BF16, tag=f"v{j}", name=f"v{j}")
                for j in range(NKC // 2)]

        # ---- input DMAs (two hardware DGE queues: sync + scalar). Each wk
        # slice rides the same queue as its x chunk so a projection matmul
        # only ever waits on ONE queue semaphore; wq/wv/wo are covered by the
        # wtouch pre-observation below. Queues are balanced so x/wq/wv all
        # land as early as possible.
        if do_dma:
            nc.sync.dma_start(out=bias_sb, in_=bias[:, :])
            for d in range(NDC):
                eng = nc.sync if d % 2 == 0 else nc.scalar
                eng.dma_start(out=wk_sb[:, d, :], in_=wk[d * P:(d + 1) * P, :])
                eng.dma_start(out=x_sb[d], in_=xT[d * P:(d + 1) * P, :])
                if d == 1:
                    nc.scalar.dma_start(out=cst_sb, in_=cst[:, :])
                if d == 2:
                    nc.sync.dma_start(
                        out=wq_sb, in_=wq.rearrange("(o p) m -> p o m", p=P))
            nc.scalar.dma_start(out=wv_sb, in_=wv.rearrange("(o p) m -> p o m", p=P))
            nc.sync.dma_start(out=wo_sb, in_=wo.rearrange("(o p) m -> p o m", p=P))
        # warm the exp table set early so the ~2.7us load overlaps the prologue
        nc.scalar.activation(out=scratch, in_=ones_col, func=Exp)
        # V ones columns, written once up front (DVE is idle in the prologue)
        for j in range(NKC // 2):
            nc.vector.tensor_copy(
                out=v_sb[j][:, :, :, HD:HD + 1].rearrange("p a b c -> p (a b c)"),
                in_=ones8)

        # Pre-observe each weight DMA on the PE with a 1x1 dummy matmul, so
        # real matmuls never need two DMA-queue waits at once (walrus can't
        # encode >1 sync wait on an LDWEIGHTS).
        wtouch_ps = ps_y.tile([1, 4], F32, tag="y", name="wtouch")
        for i, w in enumerate((wk_sb, wv_sb, wq_sb)):
            nc.tensor.matmul(wtouch_ps[:, i:i + 1],
                             lhsT=w[0:1, 0, 0:1],
                             rhs=w[0:1, 0, 0:1],
                             start=True, stop=True)
        nc.tensor.matmul(wtouch_ps[:, 3:4],
                         lhsT=wo_sb[0:1, 0, 0:1],
                         rhs=wo_sb[0:1, 0, 0:1],
                         start=True, stop=True)

        # HAM keep-warm: lowest-priority dummy matmuls that fill the PE-idle
        # stretches of the DMA-bound prologue so the clock gate stays at 8/8
        # (idle >3.4us drops the PE to half clock) when real matmuls arrive.
        with tc.high_priority(offset=-2000000):
            for i in range(36):
                warm = ps_y.tile([P, QB], F32, tag="y", name="warm")
                nc.tensor.matmul(warm[:],
                                 lhsT=x_sb[0][:, 0:P],
                                 rhs=x_sb[0][:, 0:QB],
                                 start=True, stop=True)

        # ---- projection emitters
        def emit_qk_group(w_sb, dst, bcol0, p, nb2):
            # one [128, 1024] output slab of K^T or Q^T; dst[p] [128, 2048]
            # rows 64*h2 hold head (2p+h2)'s 64 dims, columns are sequence.
            ps = ps_s.tile([P, 2, QB], F32, tag="s", name="qk_ps")
            for d in range(NDC):
                for half in range(2):
                    n0 = (2 * nb2 + half) * QB
                    nc.tensor.matmul(
                        ps[:, half],
                        lhsT=w_sb[:, d, p * P:(p + 1) * P],
                        rhs=x_sb[d][:, n0:n0 + QB],
                        start=(d == 0), stop=(d == NDC - 1))
            # evict with per-partition bias add
            with nc.allow_low_precision(reason="bf16 q/k for PE"):
                nc.scalar.activation(
                    out=dst[p][:, nb2 * 1024:(nb2 + 1) * 1024]
                        .rearrange("p (a b) -> p a b", a=2),
                    in_=ps[:],
                    func=Ident,
                    bias=bias_sb[:, bcol0 + p:bcol0 + p + 1],
                    scale=1.0)

        def emit_q_halfgroup(p, blk):
            # one [128, 512] slab of Q^T for query block blk, emitted through
            # the single-bank scratch ring so it never stalls the score ring.
            ps = ps_y.tile([P, QB], F32, tag="y", name="qh_ps")
            for d in range(NDC):
                nc.tensor.matmul(
                    ps[:],
                    lhsT=wq_sb[:, d, p * P:(p + 1) * P],
                    rhs=x_sb[d][:, blk * QB:(blk + 1) * QB],
                    start=(d == 0), stop=(d == NDC - 1))
            with nc.allow_low_precision(reason="bf16 q for PE"):
                nc.scalar.activation(
                    out=qT[p][:, blk * QB:(blk + 1) * QB],
                    in_=ps[:],
                    func=Ident,
                    bias=bias_sb[:, 0 + p:0 + p + 1],
                    scale=1.0)

        def emit_v_2chunks(j):
            # v_sb[j] [128 keys, 2, head, 65] <- chunks 2j, 2j+1
            ps = ps_y.tile([P, 2, GD], F32, tag="y", name="v_ps")
            for t in range(2):
                c = 2 * j + t
                for d in range(NDC):
                    nc.tensor.matmul(
                        ps[:, t],
                        lhsT=x_sb[d][:, c * P:(c + 1) * P],
                        rhs=wv_sb[:, d, :],
                        start=(d == 0), stop=(d == NDC - 1))
            with nc.allow_low_precision(reason="bf16 v for PE"):
                nc.vector.tensor_copy(
                    out=v_sb[j][:, :, :, 0:HD],
                    in_=ps[:].rearrange("p t (h d) -> p t h d", h=G))

        if do_proj:
            # K for pair 0 first (its matmuls start as x chunks stream in),
            # then the Q halves needed by the first two query blocks of
            # pair 0 so attention can start, then pair 1's K/Q.
            emit_qk_group(wk_sb, kT, 2, 0, 0)
            emit_qk_group(wk_sb, kT, 2, 0, 1)
            emit_qk_group(wq_sb, qT, 0, 0, 0)
            emit_qk_group(wk_sb, kT, 2, 1, 0)
            emit_qk_group(wk_sb, kT, 2, 1, 1)
            emit_qk_group(wq_sb, qT, 0, 1, 0)

        # ---- attention + output projection: per query block, head pairs
        # processed sequentially (pass p covers heads 2p, 2p+1).
        pending_outproj = None
        for qb in range(nqb if do_attn else 0):
            q0 = qb * QB
            last = (qb == nqb - 1)
            attn = []
            for p in range(2):
                av_ps = [ps_av.tile([HD + 1, QB], F32, tag="av", name="av_ps")
                         for _ in range(2)]
                for c in range(NKC):
                    if do_proj and qb == 0 and p == 0 and c % 2 == 0:
                        emit_v_2chunks(c // 2)  # V streams in ahead of its AV
                    if pending_outproj is not None and p == 0 and c == 3:
                        pending_outproj()
                        pending_outproj = None
                    # last block: pair 0's output-projection half, one m-chunk
                    # per key chunk so it rides pair 1's PE slack
                    if last and p == 1 and do_outproj and c < NDC:
                        m = c
                        up = ps_y.tile([P, QB], F32, tag="y", name="up")
                        nc.tensor.matmul(
                            up[:],
                            lhsT=wo_sb[:, 0, m * P:(m + 1) * P],
                            rhs=attn[0][:],
                            start=True, stop=True)
                        nc.vector.tensor_copy(out=u_sb[:, m, :], in_=up[:])
                    c0 = c * P
                    s_ps = ps_s.tile([P, 2, QB], F32, tag="s")
                    for h2 in range(2):
                        base = HD * h2
                        nc.tensor.matmul(
                            s_ps[:, h2],
                            lhsT=kT[p][base:base + HD, c0:c0 + P],
                            rhs=qT[p][base:base + HD, q0:q0 + QB],
                            start=True, stop=True,
                            tile_position=(base, 0))
                    pt = ptp.tile([P, 2, QB], BF16, tag="pt")
                    with nc.allow_low_precision(reason="bf16 attn weights"):
                        nc.scalar.activation(out=pt[:], in_=s_ps[:],
                                             func=Exp if do_exp else
                                             mybir.ActivationFunctionType.Copy)
                    for h2 in range(2):
                        h = 2 * p + h2
                        nc.tensor.matmul(
                            av_ps[h2][:],
                            lhsT=v_sb[c // 2][:, c % 2, h, :],
                            rhs=pt[:, h2],
                            start=(c == 0), stop=(c == NKC - 1))

                # normalize: attended^T[0:64] * (1 / sums row 64).
                # The pair's two heads are stacked into one [128, 512] tile
                # (even head written by DVE in place, odd head relocated to
                # partitions 64-127 by a small SBUF->SBUF DMA) so the output
                # projection can contract K=128 per matmul.
                at_pair = atp.tile([P, QB], BF16, tag="attn")
                for h2 in range(2):
                    if not do_norm:
                        with nc.allow_low_precision(reason="bf16 attn"):
                            nc.vector.tensor_copy(out=at_pair[0:HD, :] if h2 == 0
                                                  else at_odd,
                                                  in_=av_ps[h2][0:HD, :])
                        continue
                    av_sb = smp.tile([HD + 1, QB], F32, tag="avsb")
                    nc.vector.tensor_copy(out=av_sb, in_=av_ps[h2][:])
                    rr = tnp.tile([1, QB], F32, tag="rr")
                    nc.gpsimd.dma_start(out=rr[:, :], in_=av_sb[HD:HD + 1, :])
                    rcf = tnp.tile([1, QB], F32, tag="rcf")
                    nc.vector.reciprocal_approx_fast(out=rcf, in_=rr)
                    rc = tnp.tile([1, QB], BF16, tag="rcp")
                    with nc.allow_low_precision(reason="bf16 feed for PE bcast"):
                        nc.vector.tensor_copy(out=rc, in_=rcf)
                    bc_ps = ps_y.tile([HD, QB], F32, tag="y", name="bc_ps")
                    nc.tensor.matmul(bc_ps[:], lhsT=ones_row, rhs=rc[:],
                                     start=True, stop=True)
                    with nc.allow_low_precision(reason="bf16 attn"):
                        if h2 == 0:
                            nc.vector.tensor_tensor(out=at_pair[0:HD, :],
                                                    in0=av_sb[0:HD, :],
                                                    in1=bc_ps[:],
                                                    op=mybir.AluOpType.mult)
                        else:
                            at_odd = smp.tile([HD, QB], BF16, tag="atodd")
                            nc.vector.tensor_tensor(out=at_odd,
                                                    in0=av_sb[0:HD, :],
                                                    in1=bc_ps[:],
                                                    op=mybir.AluOpType.mult)
                            nc.gpsimd.dma_start(out=at_pair[HD:P, :],
                                                in_=at_odd[:, :])
                attn.append(at_pair)
                # remaining Q^T blocks (2 and 3), emitted through the scratch
                # ring as PE slack during block 1
                if do_proj and qb == 1:
                    with tc.high_priority(offset=-1000000):
                        emit_q_halfgroup(p, 2)
                        emit_q_halfgroup(p, 3)

            def emit_outproj(attn=attn, q0=q0):
                # y^T[m-chunk, qb] = sum_p Wo_p^T @ attn_pair_p.
                # Deprioritized: these matmuls fill PE slack so they never
                # delay the score matmuls that feed the exp stream.
                ctx2 = tc.high_priority(offset=-1000000)
                ctx2.__enter__()
                for m in range(NDC if do_outproj else 0):
                    yp = ps_y.tile([P, QB], F32, tag="y", name="yp")
                    for h in range(2):
                        nc.tensor.matmul(
                            yp[:],
                            lhsT=wo_sb[:, h, m * P:(m + 1) * P],
                            rhs=attn[h][:],
                            start=(h == 0), stop=(h == 1))
                    ysb = smp.tile([P, QB], BF16, tag="ysb")
                    with nc.allow_low_precision(reason="bf16 partial y"):
                        nc.vector.tensor_copy(out=ysb, in_=yp[:])
                    nc.sync.dma_start(out=yT[m * P:(m + 1) * P, q0:q0 + QB],
                                      in_=ysb)
                ctx2.__exit__(None, None, None)

            def emit_final(attn=attn, q0=q0):
                # last block: u_sb already holds pair 0's half. Borrow the
                # (now idle) attended PSUM ring so the matmuls, adds, and
                # DMAs of successive m-chunks pipeline.
                for m in range(NDC if do_outproj else 0):
                    yp = ps_av.tile([P, QB], F32, tag="av", name="yp")
                    nc.tensor.matmul(
                        yp[:],
                        lhsT=wo_sb[:, 1, m * P:(m + 1) * P],
                        rhs=attn[1][:],
                        start=True, stop=True)
                    ysb = smp.tile([P, QB], BF16, tag="ysb")
                    with nc.allow_low_precision(reason="bf16 partial y"):
                        nc.vector.tensor_tensor(out=ysb, in0=yp[:],
                                                in1=u_sb[:, m, :],
                                                op=mybir.AluOpType.add)
                    nc.sync.dma_start(out=yT[m * P:(m + 1) * P, q0:q0 + QB],
                                      in_=ysb)

            pending_outproj = emit_final if last else emit_outproj

        if pending_outproj is not None:
            pending_outproj()

    nc.compile()
    return nc


_CACHE = {}


def _get_nc():
    if "nc" not in _CACHE:
        _CACHE["nc"] = _build_nc()
    return _CACHE["nc"]


def make_in_maps(x, Wq, bq, Wk, bk, Wv, bv, Wo, bo):
    """Host-side sharding: per-core input dicts for cores 0..7."""
    import ml_dtypes
    bf = ml_dtypes.bfloat16
    x = np.asarray(x, np.float32)
    scale = np.float32(1.0 / np.sqrt(HD))
    Wq_s = np.asarray(Wq, np.float32) * scale
    bq_s = np.asarray(bq, np.float32) * scale
    Wk = np.asarray(Wk, np.float32)
    bk = np.asarray(bk, np.float32)
    Wv = np.asarray(Wv, np.float32)
    Wo = np.asarray(Wo, np.float32)

    C = np.zeros((P, 768), bf)
    C[:, 0] = 1.0
    C[0, 641:641 + HD] = 1.0
    C[:, 740:748] = 1.0

    xts = [np.ascontiguousarray(x[b].T.astype(bf)) for b in range(2)]
    in_maps = []
    for core in range(8):
        b, g = divmod(core, 4)
        cols = slice(g * GD, (g + 1) * GD)
        bias = np.zeros((P, 4), np.float32)
        bias[:, 0] = bq_s[g * GD:g * GD + P]
        bias[:, 1] = bq_s[g * GD + P:(g + 1) * GD]
        bias[:, 2] = bk[g * GD:g * GD + P]
        bias[:, 3] = bk[g * GD + P:(g + 1) * GD]
        in_maps.append({
            "cst": C,
            "xT": xts[b],
            "wq": np.ascontiguousarray(Wq_s[:, cols].astype(bf)),
            "wk": np.ascontiguousarray(Wk[:, cols].astype(bf)),
            "wv": np.ascontiguousarray(Wv[:, cols].astype(bf)),
            "wo": np.ascontiguousarray(Wo[cols, :].astype(bf)),
            "bias": bias,
        })
    return in_maps


def gather_output(results, Wv, bv, Wo, bo):
    """Sum per-core partial y^T outputs and fold bv/bo exactly."""
    y = np.zeros((2, S, D), np.float32)
    for core in range(8):
        b = core // 4
        y[b] += results[core]["yT"].T.astype(np.float32)
    y += np.asarray(bo, np.float32) + np.asarray(bv, np.float32) @ np.asarray(Wo, np.float32)
    return y


def kernel(x, Wq, bq, Wk, bk, Wv, bv, Wo, bo):
    global LAST_RESULTS
    from concourse.bass_utils import run_bass_kernel_spmd
    in_maps = make_in_maps(x, Wq, bq, Wk, bk, Wv, bv, Wo, bo)
    res = run_bass_kernel_spmd(_get_nc(), in_maps, core_ids=list(range(8)),
                               trace=TRACE)
    LAST_RESULTS = res
    return gather_output(res.results, Wv, bv, Wo, bo)


# revision 23
# speedup vs baseline: 1.0637x; 1.0637x over previous
"""Trainium2 Bass kernel for a 16-head self-attention block.

Model (matches the nn.Module reference):
    q = x @ Wq + bq; k = x @ Wk + bk; v = x @ Wv + bv   (per-head split, Hd=64)
    attn = softmax(q k^T / sqrt(Hd)); out = (attn v) @ Wo + bo
Shapes: x [2, 2048, 1024], 16 heads, head dim 64.

Sharding (8 cores): core = (batch b in {0,1}) x (head-group g in {0..3});
each core owns 4 heads of one batch element. Inputs are sliced on the host;
each core returns a partial y^T = (attended_g @ Wo_g)^T which the host sums
over the 4 head-groups per batch.

Per-core design (bf16 matmul operands, fp32 PSUM accumulation — fp32r runs
fp32_mode=HIGH two-pass on hardware, ~2x the bf16 stream time):
  - Host passes xT = x[b]^T so projections need no on-device transpose.
  - Scores are computed transposed, S^T[key, q] = K_h Q_h^T, so softmax's
    exp runs straight out of PSUM on the Scalar engine and A = P V consumes
    P^T with no transpose anywhere. The two heads of a pair run as
    CONCURRENT row-group matmuls (K=64 at PE row offsets 0/64), so a score
    pair costs ~one 512-row pass.
  - softmax skips the max subtraction (mathematically identical; scores are
    O(5) here and ACT exp is <=2 ULP on [-10,10]).
  - P row sums come from a ones column appended to V (A' = P [V|1]), so the
    A.V matmul (M=65) yields attended^T plus the sums row with no extra
    matmuls or PSUM banks.
  - Normalization: sums row -> partition 0 via tiny SBUF->SBUF DMA, fast
    approx reciprocal (+cast to bf16), broadcast to 64 rows with a K=1 ones
    outer-product on the PE, then one vector multiply per head.
  - 1/sqrt(Hd) is folded into Wq (and bq) on the host; bv and bo are folded
    in exactly on the host: y += bo + bv @ Wo (softmax rows sum to 1).

Scheduling notes (exp stream is the critical resource, ~1.15us per key
chunk; everything else must fit underneath it):
  - PSUM banks: scores 2x[128,2,512] (4) + attended 3x[65,512] (3) +
    scratch ring 1x[128,512] (1) = 8. Three attended bufs let the next
    pair's first A.V start before the previous pair's PSUM eviction.
  - V is projected just-in-time during block 0 pair 0, two key chunks per
    PSUM tile.
  - The Q^T halves for blocks 2/3 are emitted during block 1 as
    single-bank groups in the scratch ring (not the score ring, which
    would stall the exp stream behind their deprioritized eviction).
  - Output projection of block qb is emitted a few chunks into block qb+1
    (PE slack). For the LAST block, pair 0's half runs during pair 1's
    c-loop into u_sb, and the tail only runs pair 1's matmuls + an add.
  - Input DMAs are split across the two hardware DGE queues (sync + ACT)
    and wk/wq are sliced per contraction chunk so the first projection
    matmuls start as soon as x chunk 0 lands.
"""

import numpy as np

import concourse.bass as bass
import concourse.tile as tile
from concourse import bacc
from concourse import mybir

P = 128          # partitions
S = 2048         # sequence length
D = 1024         # model dim
H = 16           # total heads
HD = 64          # head dim
G = 4            # heads per core
GD = G * HD      # 256 head-group dims per core
NQB = 4          # query blocks
QB = S // NQB    # 512
NKC = S // P     # 16 key chunks
NDC = D // P     # 8 contraction chunks
F32 = mybir.dt.float32
BF16 = mybir.dt.bfloat16

TRACE = False
LAST_RESULTS = None


def _build_nc(nqb=NQB, do_attn=True, do_exp=True, do_outproj=True,
              do_norm=True, do_proj=True, do_dma=True):
    nc = bacc.Bacc(trn_type="TRN2")
    xT = nc.dram_tensor("xT", [D, S], BF16, kind="ExternalInput")
    wq = nc.dram_tensor("wq", [D, GD], BF16, kind="ExternalInput")
    wk = nc.dram_tensor("wk", [D, GD], BF16, kind="ExternalInput")
    wv = nc.dram_tensor("wv", [D, GD], BF16, kind="ExternalInput")
    wo = nc.dram_tensor("wo", [GD, D], BF16, kind="ExternalInput")
    bias = nc.dram_tensor("bias", [P, 4], F32, kind="ExternalInput")
    cst = nc.dram_tensor("cst", [P, 768], BF16, kind="ExternalInput")
    yT = nc.dram_tensor("yT", [D, S], BF16, kind="ExternalOutput")

    Exp = mybir.ActivationFunctionType.Exp
    Ident = mybir.ActivationFunctionType.Identity

    with tile.TileContext(nc) as tc, \
         tc.tile_pool(name="sb", bufs=1) as sb, \
         tc.tile_pool(name="pt", bufs=3) as ptp, \
         tc.tile_pool(name="small", bufs=2) as smp, \
         tc.tile_pool(name="attnp", bufs=5) as atp, \
         tc.tile_pool(name="tiny", bufs=2) as tnp, \
         tc.tile_pool(name="ps_s", bufs=2, space="PSUM") as ps_s, \
         tc.tile_pool(name="ps_av", bufs=3, space="PSUM") as ps_av, \
         tc.tile_pool(name="ps_y", bufs=1, space="PSUM") as ps_y:

        # ---- persistent SBUF tensors
        wq_sb = sb.tile([P, NDC, GD], BF16, tag="wq")
        wk_sb = sb.tile([P, NDC, GD], BF16, tag="wk")
        wv_sb = sb.tile([P, NDC, GD], BF16, tag="wv")
        wo_sb = sb.tile([P, 2, D], BF16, tag="wo")   # [pair-dims, pair, out-dim]
        bias_sb = sb.tile([P, 4], F32, tag="bias")
        scratch = sb.tile([P, 1], F32, tag="scratch")
        cst_sb = sb.tile([P, 768], BF16, tag="cst")
        ones_col = cst_sb[:, 0:1]             # [128, 1] ones
        ones_row = cst_sb[0:1, 641:641 + HD]  # [1, 64] ones
        ones8 = cst_sb[:, 740:748]            # [128, 8] ones
        u_sb = sb.tile([P, NDC, QB], F32, tag="u")   # last-block pair-0 y half
        x_sb = [sb.tile([P, S], BF16, tag=f"x{d}", name=f"x{d}") for d in range(NDC)]
        kT = [sb.tile([P, S], BF16, tag=f"k{p}", name=f"k{p}") for p in range(2)]
        qT = [sb.tile([P, S], BF16, tag=f"q{p}", name=f"q{p}") for p in range(2)]
        # V with a ones column per head, two key chunks per tile:
        # [keys, chunk-pair half, head, 65]
        v_sb = [sb.tile([P, 2, G, HD + 1], BF16, tag=f"v{j}", name=f"v{j}")
                for j in range(NKC // 2)]

        # ---- input DMAs (two hardware DGE queues: sync + scalar). Each wk
        # slice rides the same queue as its x chunk so a projection matmul
        # only ever waits on ONE queue semaphore; wq/wv/wo are covered by the
        # wtouch pre-observation below. Queues are balanced so x/wq/wv all
        # land as early as possible.
        if do_dma:
            nc.sync.dma_start(out=bias_sb, in_=bias[:, :])
            for d in range(NDC):
                eng = nc.sync if d % 2 == 0 else nc.scalar
                eng.dma_start(out=wk_sb[:, d, :], in_=wk[d * P:(d + 1) * P, :])
                eng.dma_start(out=x_sb[d], in_=xT[d * P:(d + 1) * P, :])
                if d == 1:
                    nc.scalar.dma_start(out=cst_sb, in_=cst[:, :])
            nc.sync.dma_start(out=wq_sb, in_=wq.rearrange("(o p) m -> p o m", p=P))
            nc.scalar.dma_start(out=wv_sb, in_=wv.rearrange("(o p) m -> p o m", p=P))
            nc.sync.dma_start(out=wo_sb, in_=wo.rearrange("(o p) m -> p o m", p=P))
        # warm the exp table set early so the ~2.7us load overlaps the prologue
        nc.scalar.activation(out=scratch, in_=ones_col, func=Exp)
        # V ones columns, written once up front (DVE is idle in the prologue)
        for j in range(NKC // 2):
            nc.vector.tensor_copy(
                out=v_sb[j][:, :, :, HD:HD + 1].rearrange("p a b c -> p (a b c)"),
                in_=ones8)

        # Pre-observe each weight DMA on the PE with a 1x1 dummy matmul, so
        # real matmuls never need two DMA-queue waits at once (walrus can't
        # encode >1 sync wait on an LDWEIGHTS).
        wtouch_ps = ps_y.tile([1, 4], F32, tag="y", name="wtouch")
        for i, w in enumerate((wk_sb, wv_sb, wq_sb)):
            nc.tensor.matmul(wtouch_ps[:, i:i + 1],
                             lhsT=w[0:1, 0, 0:1],
                             rhs=w[0:1, 0, 0:1],
                             start=True, stop=True)
        nc.tensor.matmul(wtouch_ps[:, 3:4],
                         lhsT=wo_sb[0:1, 0, 0:1],
                         rhs=wo_sb[0:1, 0, 0:1],
                         start=True, stop=True)

        # ---- projection emitters
        def emit_qk_group(w_sb, dst, bcol0, p, nb2):
            # one [128, 1024] output slab of K^T or Q^T; dst[p] [128, 2048]
            # rows 64*h2 hold head (2p+h2)'s 64 dims, columns are sequence.
            ps = ps_s.tile([P, 2, QB], F32, tag="s", name="qk_ps")
            for d in range(NDC):
                for half in range(2):
                    n0 = (2 * nb2 + half) * QB
                    nc.tensor.matmul(
                        ps[:, half],
                        lhsT=w_sb[:, d, p * P:(p + 1) * P],
                        rhs=x_sb[d][:, n0:n0 + QB],
                        start=(d == 0), stop=(d == NDC - 1))
            # evict with per-partition bias add
            with nc.allow_low_precision(reason="bf16 q/k for PE"):
                nc.scalar.activation(
                    out=dst[p][:, nb2 * 1024:(nb2 + 1) * 1024]
                        .rearrange("p (a b) -> p a b", a=2),
                    in_=ps[:],
                    func=Ident,
                    bias=bias_sb[:, bcol0 + p:bcol0 + p + 1],
                    scale=1.0)

        def emit_q_halfgroup(p, blk):
            # one [128, 512] slab of Q^T for query block blk, emitted through
            # the single-bank scratch ring so it never stalls the score ring.
            ps = ps_y.tile([P, QB], F32, tag="y", name="qh_ps")
            for d in range(NDC):
                nc.tensor.matmul(
                    ps[:],
                    lhsT=wq_sb[:, d, p * P:(p + 1) * P],
                    rhs=x_sb[d][:, blk * QB:(blk + 1) * QB],
                    start=(d == 0), stop=(d == NDC - 1))
            with nc.allow_low_precision(reason="bf16 q for PE"):
                nc.scalar.activation(
                    out=qT[p][:, blk * QB:(blk + 1) * QB],
                    in_=ps[:],
                    func=Ident,
                    bias=bias_sb[:, 0 + p:0 + p + 1],
                    scale=1.0)

        def emit_v_2chunks(j):
            # v_sb[j] [128 keys, 2, head, 65] <- chunks 2j, 2j+1
            ps = ps_y.tile([P, 2, GD], F32, tag="y", name="v_ps")
            for t in range(2):
                c = 2 * j + t
                for d in range(NDC):
                    nc.tensor.matmul(
                        ps[:, t],
                        lhsT=x_sb[d][:, c * P:(c + 1) * P],
                        rhs=wv_sb[:, d, :],
                        start=(d == 0), stop=(d == NDC - 1))
            with nc.allow_low_precision(reason="bf16 v for PE"):
                nc.vector.tensor_copy(
                    out=v_sb[j][:, :, :, 0:HD],
                    in_=ps[:].rearrange("p t (h d) -> p t h d", h=G))

        if do_proj:
            # K for pair 0 first (its matmuls start as x chunks stream in),
            # then the Q halves needed by the first two query blocks of
            # pair 0 so attention can start, then pair 1's K/Q.
            emit_qk_group(wk_sb, kT, 2, 0, 0)
            emit_qk_group(wk_sb, kT, 2, 0, 1)
            emit_qk_group(wq_sb, qT, 0, 0, 0)
            emit_qk_group(wk_sb, kT, 2, 1, 0)
            emit_qk_group(wk_sb, kT, 2, 1, 1)
            emit_qk_group(wq_sb, qT, 0, 1, 0)

        # ---- attention + output projection: per query block, head pairs
        # processed sequentially (pass p covers heads 2p, 2p+1).
        pending_outproj = None
        pend_norm = None
        for qb in range(nqb if do_attn else 0):
            q0 = qb * QB
            last = (qb == nqb - 1)
            attn = []
            for p in range(2):
                av_ps = [ps_av.tile([HD + 1, QB], F32, tag="av", name="av_ps")
                         for _ in range(2)]
                for c in range(NKC):
                    if do_proj and qb == 0 and p == 0 and c % 2 == 0:
                        emit_v_2chunks(c // 2)  # V streams in ahead of its AV
                    if pend_norm is not None and c == 2:
                        pend_norm()
                        pend_norm = None
                    if pending_outproj is not None and p == 0 and c == 5:
                        pending_outproj()
                        pending_outproj = None
                    # last block: pair 0's output-projection half, one m-chunk
                    # per key chunk so it rides pair 1's PE slack
                    if last and p == 1 and do_outproj and 5 <= c < 5 + NDC:
                        m = c - 5
                        up = ps_y.tile([P, QB], F32, tag="y", name="up")
                        nc.tensor.matmul(
                            up[:],
                            lhsT=wo_sb[:, 0, m * P:(m + 1) * P],
                            rhs=attn[0][:],
                            start=True, stop=True)
                        nc.vector.tensor_copy(out=u_sb[:, m, :], in_=up[:])
                    c0 = c * P
                    s_ps = ps_s.tile([P, 2, QB], F32, tag="s")
                    for h2 in range(2):
                        base = HD * h2
                        nc.tensor.matmul(
                            s_ps[:, h2],
                            lhsT=kT[p][base:base + HD, c0:c0 + P],
                            rhs=qT[p][base:base + HD, q0:q0 + QB],
                            start=True, stop=True,
                            tile_position=(base, 0))
                    pt = ptp.tile([P, 2, QB], BF16, tag="pt")
                    with nc.allow_low_precision(reason="bf16 attn weights"):
                        nc.scalar.activation(out=pt[:], in_=s_ps[:],
                                             func=Exp if do_exp else
                                             mybir.ActivationFunctionType.Copy)
                    for h2 in range(2):
                        h = 2 * p + h2
                        nc.tensor.matmul(
                            av_ps[h2][:],
                            lhsT=v_sb[c // 2][:, c % 2, h, :],
                            rhs=pt[:, h2],
                            start=(c == 0), stop=(c == NKC - 1))

                # normalize stage 1 (immediate; DVE/DMA only): evict the
                # pair's attended^T + sums PSUM, 1/sums via fast reciprocal.
                # Stage 2 (the PE ones-broadcast + multiply + odd-head
                # relocation) is DEFERRED into the next pass's c-loop: every
                # matmul's completion feeds one global PE counter that later
                # consumers wait on, so a broadcast matmul parked on the
                # reciprocal chain at a pass boundary would stall the next
                # pass's scores — and with them the exp stream.
                at_pair = atp.tile([P, QB], BF16, tag="attn")
                av_sbs, rcs = [], []
                for h2 in range(2):
                    av_sb = smp.tile([HD + 1, QB], F32, tag="avsb")
                    nc.vector.tensor_copy(out=av_sb, in_=av_ps[h2][:])
                    av_sbs.append(av_sb)
                    if not do_norm:
                        continue
                    rr = tnp.tile([1, QB], F32, tag="rr")
                    nc.gpsimd.dma_start(out=rr[:, :], in_=av_sb[HD:HD + 1, :])
                    rcf = tnp.tile([1, QB], F32, tag="rcf")
                    nc.vector.reciprocal_approx_fast(out=rcf, in_=rr)
                    rc = tnp.tile([1, QB], BF16, tag="rcp")
                    with nc.allow_low_precision(reason="bf16 feed for PE bcast"):
                        nc.vector.tensor_copy(out=rc, in_=rcf)
                    rcs.append(rc)

                def pending_norm(at_pair=at_pair, av_sbs=av_sbs, rcs=rcs):
                    for h2 in range(2):
                        if not do_norm:
                            with nc.allow_low_precision(reason="bf16 attn"):
                                if h2 == 0:
                                    nc.vector.tensor_copy(
                                        out=at_pair[0:HD, :],
                                        in_=av_sbs[0][0:HD, :])
                                else:
                                    at_odd = smp.tile([HD, QB], BF16,
                                                      tag="atodd")
                                    nc.vector.tensor_copy(
                                        out=at_odd, in_=av_sbs[1][0:HD, :])
                                    nc.gpsimd.dma_start(out=at_pair[HD:P, :],
                                                        in_=at_odd[:, :])
                            continue
                        bc_ps = ps_y.tile([HD, QB], F32, tag="y", name="bc_ps")
                        nc.tensor.matmul(bc_ps[:], lhsT=ones_row,
                                         rhs=rcs[h2][:], start=True, stop=True)
                        with nc.allow_low_precision(reason="bf16 attn"):
                            if h2 == 0:
                                nc.vector.tensor_tensor(out=at_pair[0:HD, :],
                                                        in0=av_sbs[0][0:HD, :],
                                                        in1=bc_ps[:],
                                                        op=mybir.AluOpType.mult)
                            else:
                                at_odd = smp.tile([HD, QB], BF16, tag="atodd")
                                nc.vector.tensor_tensor(out=at_odd,
                                                        in0=av_sbs[1][0:HD, :],
                                                        in1=bc_ps[:],
                                                        op=mybir.AluOpType.mult)
                                nc.gpsimd.dma_start(out=at_pair[HD:P, :],
                                                    in_=at_odd[:, :])
                attn.append(at_pair)
                pend_norm = pending_norm
                # remaining Q^T blocks (2 and 3), emitted through the scratch
                # ring as PE slack during block 1
                if do_proj and qb == 1:
                    with tc.high_priority(offset=-1000000):
                        emit_q_halfgroup(p, 2)
                        emit_q_halfgroup(p, 3)

            def emit_outproj(attn=attn, q0=q0):
                # y^T[m-chunk, qb] = sum_p Wo_p^T @ attn_pair_p.
                # Deprioritized: these matmuls fill PE slack so they never
                # delay the score matmuls that feed the exp stream.
                ctx2 = tc.high_priority(offset=-1000000)
                ctx2.__enter__()
                for m in range(NDC if do_outproj else 0):
                    yp = ps_y.tile([P, QB], F32, tag="y", name="yp")
                    for h in range(2):
                        nc.tensor.matmul(
                            yp[:],
                            lhsT=wo_sb[:, h, m * P:(m + 1) * P],
                            rhs=attn[h][:],
                            start=(h == 0), stop=(h == 1))
                    ysb = smp.tile([P, QB], BF16, tag="ysb")
                    with nc.allow_low_precision(reason="bf16 partial y"):
                        nc.vector.tensor_copy(out=ysb, in_=yp[:])
                    nc.sync.dma_start(out=yT[m * P:(m + 1) * P, q0:q0 + QB],
                                      in_=ysb)
                ctx2.__exit__(None, None, None)

            def emit_final(attn=attn, q0=q0):
                # last block: u_sb already holds pair 0's half. Borrow the
                # (now idle) attended PSUM ring so the matmuls, adds, and
                # DMAs of successive m-chunks pipeline.
                for m in range(NDC if do_outproj else 0):
                    yp = ps_av.tile([P, QB], F32, tag="av", name="yp")
                    nc.tensor.matmul(
                        yp[:],
                        lhsT=wo_sb[:, 1, m * P:(m + 1) * P],
                        rhs=attn[1][:],
                        start=True, stop=True)
                    ysb = smp.tile([P, QB], BF16, tag="ysb")
                    with nc.allow_low_precision(reason="bf16 partial y"):
                        nc.vector.tensor_tensor(out=ysb, in0=yp[:],
                                                in1=u_sb[:, m, :],
                                                op=mybir.AluOpType.add)
                    nc.sync.dma_start(out=yT[m * P:(m + 1) * P, q0:q0 + QB],
                                      in_=ysb)

            pending_outproj = emit_final if last else emit_outproj

        if pend_norm is not None:
            pend_norm()
        if pending_outproj is not None:
            pending_outproj()

    nc.compile()
    return nc


_CACHE = {}


def _get_nc():
    if "nc" not in _CACHE:
        _CACHE["nc"] = _build_nc()
    return _CACHE["nc"]


def make_in_maps(x, Wq, bq, Wk, bk, Wv, bv, Wo, bo):
    """Host-side sharding: per-core input dicts for cores 0..7."""
    import ml_dtypes
    bf = ml_dtypes.bfloat16
    x = np.asarray(x, np.float32)
    scale = np.float32(1.0 / np.sqrt(HD))
    Wq_s = np.asarray(Wq, np.float32) * scale
    bq_s = np.asarray(bq, np.float32) * scale
    Wk = np.asarray(Wk, np.float32)
    bk = np.asarray(bk, np.float32)
    Wv = np.asarray(Wv, np.float32)
    Wo = np.asarray(Wo, np.float32)

    C = np.zeros((P, 768), bf)
    C[:, 0] = 1.0
    C[0, 641:641 + HD] = 1.0
    C[:, 740:748] = 1.0

    xts = [np.ascontiguousarray(x[b].T.astype(bf)) for b in range(2)]
    in_maps = []
    for core in range(8):
        b, g = divmod(core, 4)
        cols = slice(g * GD, (g + 1) * GD)
        bias = np.zeros((P, 4), np.float32)
        bias[:, 0] = bq_s[g * GD:g * GD + P]
        bias[:, 1] = bq_s[g * GD + P:(g + 1) * GD]
        bias[:, 2] = bk[g * GD:g * GD + P]
        bias[:, 3] = bk[g * GD + P:(g + 1) * GD]
        in_maps.append({
            "cst": C,
            "xT": xts[b],
            "wq": np.ascontiguousarray(Wq_s[:, cols].astype(bf)),
            "wk": np.ascontiguousarray(Wk[:, cols].astype(bf)),
            "wv": np.ascontiguousarray(Wv[:, cols].astype(bf)),
            "wo": np.ascontiguousarray(Wo[cols, :].astype(bf)),
            "bias": bias,
        })
    return in_maps


def gather_output(results, Wv, bv, Wo, bo):
    """Sum per-core partial y^T outputs and fold bv/bo exactly."""
    y = np.zeros((2, S, D), np.float32)
    for core in range(8):
        b = core // 4
        y[b] += results[core]["yT"].T.astype(np.float32)
    y += np.asarray(bo, np.float32) + np.asarray(bv, np.float32) @ np.asarray(Wo, np.float32)
    return y


def kernel(x, Wq, bq, Wk, bk, Wv, bv, Wo, bo):
    global LAST_RESULTS
    from concourse.bass_utils import run_bass_kernel_spmd
    in_maps = make_in_maps(x, Wq, bq, Wk, bk, Wv, bv, Wo, bo)
    res = run_bass_kernel_spmd(_get_nc(), in_maps, core_ids=list(range(8)),
                               trace=TRACE)
    LAST_RESULTS = res
    return gather_output(res.results, Wv, bv, Wo, bo)


# revision 26
# speedup vs baseline: 1.0704x; 1.0064x over previous
"""Trainium2 Bass kernel for a 16-head self-attention block.

Model (matches the nn.Module reference):
    q = x @ Wq + bq; k = x @ Wk + bk; v = x @ Wv + bv   (per-head split, Hd=64)
    attn = softmax(q k^T / sqrt(Hd)); out = (attn v) @ Wo + bo
Shapes: x [2, 2048, 1024], 16 heads, head dim 64.

Sharding (8 cores): core = (batch b in {0,1}) x (head-group g in {0..3});
each core owns 4 heads of one batch element. Inputs are sliced on the host;
each core returns a partial y^T = (attended_g @ Wo_g)^T which the host sums
over the 4 head-groups per batch.

Per-core design (bf16 matmul operands, fp32 PSUM accumulation — fp32r runs
fp32_mode=HIGH two-pass on hardware, ~2x the bf16 stream time):
  - Host passes xT = x[b]^T so projections need no on-device transpose.
  - Scores are computed transposed, S^T[key, q] = K_h Q_h^T, so softmax's
    exp runs straight out of PSUM on the Scalar engine and A = P V consumes
    P^T with no transpose anywhere. The two heads of a pair run as
    CONCURRENT row-group matmuls (K=64 at PE row offsets 0/64), so a score
    pair costs ~one 512-row pass.
  - softmax skips the max subtraction (mathematically identical; scores are
    O(5) here and ACT exp is <=2 ULP on [-10,10]).
  - P row sums come from a ones column appended to V (A' = P [V|1]), so the
    A.V matmul (M=65) yields attended^T plus the sums row with no extra
    matmuls or PSUM banks.
  - Normalization: sums row -> partition 0 via tiny SBUF->SBUF DMA, fast
    approx reciprocal (+cast to bf16), broadcast to 64 rows with a K=1 ones
    outer-product on the PE, then one vector multiply per head.
  - 1/sqrt(Hd) is folded into Wq (and bq) on the host; bv and bo are folded
    in exactly on the host: y += bo + bv @ Wo (softmax rows sum to 1).

Scheduling notes (exp stream is the critical resource, ~1.15us per key
chunk; everything else must fit underneath it):
  - PSUM banks: scores 2x[128,2,512] (4) + attended 3x[65,512] (3) +
    scratch ring 1x[128,512] (1) = 8. Three attended bufs let the next
    pair's first A.V start before the previous pair's PSUM eviction.
  - V is projected just-in-time during block 0 pair 0, two key chunks per
    PSUM tile.
  - The Q^T halves for blocks 2/3 are emitted during block 1 as
    single-bank groups in the scratch ring (not the score ring, which
    would stall the exp stream behind their deprioritized eviction).
  - Output projection of block qb is emitted a few chunks into block qb+1
    (PE slack). For the LAST block, pair 0's half runs during pair 1's
    c-loop into u_sb, and the tail only runs pair 1's matmuls + an add.
  - Input DMAs are split across the two hardware DGE queues (sync + ACT)
    and wk/wq are sliced per contraction chunk so the first projection
    matmuls start as soon as x chunk 0 lands.
"""

import numpy as np

import concourse.bass as bass
import concourse.tile as tile
from concourse import bacc
from concourse import mybir

P = 128          # partitions
S = 2048         # sequence length
D = 1024         # model dim
H = 16           # total heads
HD = 64          # head dim
G = 4            # heads per core
GD = G * HD      # 256 head-group dims per core
NQB = 4          # query blocks
QB = S // NQB    # 512
NKC = S // P     # 16 key chunks
NDC = D // P     # 8 contraction chunks
F32 = mybir.dt.float32
BF16 = mybir.dt.bfloat16

TRACE = False
LAST_RESULTS = None


def _build_nc(nqb=NQB, do_attn=True, do_exp=True, do_outproj=True,
              do_norm=True, do_proj=True, do_dma=True):
    nc = bacc.Bacc(trn_type="TRN2")
    xT = nc.dram_tensor("xT", [D, S], BF16, kind="ExternalInput")
    wq = nc.dram_tensor("wq", [D, GD], BF16, kind="ExternalInput")
    wk = nc.dram_tensor("wk", [D, GD], BF16, kind="ExternalInput")
    wv = nc.dram_tensor("wv", [D, GD], BF16, kind="ExternalInput")
    wo = nc.dram_tensor("wo", [GD, D], BF16, kind="ExternalInput")
    bias = nc.dram_tensor("bias", [P, 4], F32, kind="ExternalInput")
    cst = nc.dram_tensor("cst", [P, 768], BF16, kind="ExternalInput")
    yT = nc.dram_tensor("yT", [D, S], BF16, kind="ExternalOutput")

    Exp = mybir.ActivationFunctionType.Exp
    Ident = mybir.ActivationFunctionType.Identity

    with tile.TileContext(nc) as tc, \
         tc.tile_pool(name="sb", bufs=1) as sb, \
         tc.tile_pool(name="pt", bufs=3) as ptp, \
         tc.tile_pool(name="small", bufs=2) as smp, \
         tc.tile_pool(name="attnp", bufs=5) as atp, \
         tc.tile_pool(name="tiny", bufs=2) as tnp, \
         tc.tile_pool(name="ps_s", bufs=2, space="PSUM") as ps_s, \
         tc.tile_pool(name="ps_av", bufs=3, space="PSUM") as ps_av, \
         tc.tile_pool(name="ps_y", bufs=1, space="PSUM") as ps_y:

        # ---- persistent SBUF tensors
        wq_sb = sb.tile([P, NDC, GD], BF16, tag="wq")
        wk_sb = sb.tile([P, NDC, GD], BF16, tag="wk")
        wv_sb = sb.tile([P, NDC, GD], BF16, tag="wv")
        wo_sb = sb.tile([P, 2, D], BF16, tag="wo")   # [pair-dims, pair, out-dim]
        bias_sb = sb.tile([P, 4], F32, tag="bias")
        scratch = sb.tile([P, 1], F32, tag="scratch")
        cst_sb = sb.tile([P, 768], BF16, tag="cst")
        ones_col = cst_sb[:, 0:1]             # [128, 1] ones
        ones_row = cst_sb[0:1, 641:641 + HD]  # [1, 64] ones
        ones8 = cst_sb[:, 740:748]            # [128, 8] ones
        u_sb = sb.tile([P, NDC, QB], F32, tag="u")   # last-block pair-0 y half
        x_sb = [sb.tile([P, S], BF16, tag=f"x{d}", name=f"x{d}") for d in range(NDC)]
        kT = [sb.tile([P, S], BF16, tag=f"k{p}", name=f"k{p}") for p in range(2)]
        qT = [sb.tile([P, S], BF16, tag=f"q{p}", name=f"q{p}") for p in range(2)]
        # V with a ones column per head, two key chunks per tile:
        # [keys, chunk-pair half, head, 65]
        v_sb = [sb.tile([P, 2, G, HD + 1], BF16, tag=f"v{j}", name=f"v{j}")
                for j in range(NKC // 2)]

        # ---- input DMAs (two hardware DGE queues: sync + scalar). Each wk
        # slice rides the same queue as its x chunk so a projection matmul
        # only ever waits on ONE queue semaphore; wq/wv/wo are covered by the
        # wtouch pre-observation below. Queues are balanced so x/wq/wv all
        # land as early as possible.
        if do_dma:
            nc.sync.dma_start(out=bias_sb, in_=bias[:, :])
            for d in range(NDC):
                eng = nc.sync if d % 2 == 0 else nc.scalar
                eng.dma_start(out=wk_sb[:, d, :], in_=wk[d * P:(d + 1) * P, :])
                eng.dma_start(out=x_sb[d], in_=xT[d * P:(d + 1) * P, :])
                if d == 1:
                    nc.scalar.dma_start(out=cst_sb, in_=cst[:, :])
            nc.sync.dma_start(out=wq_sb, in_=wq.rearrange("(o p) m -> p o m", p=P))
            nc.scalar.dma_start(out=wv_sb, in_=wv.rearrange("(o p) m -> p o m", p=P))
            nc.sync.dma_start(out=wo_sb, in_=wo.rearrange("(o p) m -> p o m", p=P))
        # warm the exp table set early so the ~2.7us load overlaps the prologue
        nc.scalar.activation(out=scratch, in_=ones_col, func=Exp)
        # V ones columns, written once up front (DVE is idle in the prologue)
        for j in range(NKC // 2):
            nc.vector.tensor_copy(
                out=v_sb[j][:, :, :, HD:HD + 1].rearrange("p a b c -> p (a b c)"),
                in_=ones8)

        # Pre-observe each weight DMA on the PE with a 1x1 dummy matmul, so
        # real matmuls never need two DMA-queue waits at once (walrus can't
        # encode >1 sync wait on an LDWEIGHTS).
        wtouch_ps = ps_y.tile([1, 4], F32, tag="y", name="wtouch")
        for i, w in enumerate((wk_sb, wv_sb, wq_sb)):
            nc.tensor.matmul(wtouch_ps[:, i:i + 1],
                             lhsT=w[0:1, 0, 0:1],
                             rhs=w[0:1, 0, 0:1],
                             start=True, stop=True)
        nc.tensor.matmul(wtouch_ps[:, 3:4],
                         lhsT=wo_sb[0:1, 0, 0:1],
                         rhs=wo_sb[0:1, 0, 0:1],
                         start=True, stop=True)

        # ---- projection emitters
        def emit_qk_group(w_sb, dst, bcol0, p, nb2):
            # one [128, 1024] output slab of K^T or Q^T; dst[p] [128, 2048]
            # rows 64*h2 hold head (2p+h2)'s 64 dims, columns are sequence.
            ps = ps_s.tile([P, 2, QB], F32, tag="s", name="qk_ps")
            for d in range(NDC):
                for half in range(2):
                    n0 = (2 * nb2 + half) * QB
                    nc.tensor.matmul(
                        ps[:, half],
                        lhsT=w_sb[:, d, p * P:(p + 1) * P],
                        rhs=x_sb[d][:, n0:n0 + QB],
                        start=(d == 0), stop=(d == NDC - 1))
            # evict with per-partition bias add
            with nc.allow_low_precision(reason="bf16 q/k for PE"):
                nc.scalar.activation(
                    out=dst[p][:, nb2 * 1024:(nb2 + 1) * 1024]
                        .rearrange("p (a b) -> p a b", a=2),
                    in_=ps[:],
                    func=Ident,
                    bias=bias_sb[:, bcol0 + p:bcol0 + p + 1],
                    scale=1.0)

        def emit_q_halfgroup(p, blk):
            # one [128, 512] slab of Q^T for query block blk. Rides the
            # attended ring's third slot (free mid-pass) and evicts on the
            # Vector engine, so neither the score ring nor the exp stream
            # ever waits on its lazy, slack-scheduled matmuls.
            ps = ps_av.tile([P, QB], F32, tag="av", name="qh_ps")
            for d in range(NDC):
                nc.tensor.matmul(
                    ps[:],
                    lhsT=wq_sb[:, d, p * P:(p + 1) * P],
                    rhs=x_sb[d][:, blk * QB:(blk + 1) * QB],
                    start=(d == 0), stop=(d == NDC - 1))
            with nc.allow_low_precision(reason="bf16 q for PE"):
                nc.vector.tensor_scalar_add(
                    out=qT[p][:, blk * QB:(blk + 1) * QB],
                    in0=ps[:],
                    scalar1=bias_sb[:, 0 + p:0 + p + 1])

        def emit_v_2chunks(j):
            # v_sb[j] [128 keys, 2, head, 65] <- chunks 2j, 2j+1
            ps = ps_y.tile([P, 2, GD], F32, tag="y", name="v_ps")
            for t in range(2):
                c = 2 * j + t
                for d in range(NDC):
                    nc.tensor.matmul(
                        ps[:, t],
                        lhsT=x_sb[d][:, c * P:(c + 1) * P],
                        rhs=wv_sb[:, d, :],
                        start=(d == 0), stop=(d == NDC - 1))
            with nc.allow_low_precision(reason="bf16 v for PE"):
                nc.vector.tensor_copy(
                    out=v_sb[j][:, :, :, 0:HD],
                    in_=ps[:].rearrange("p t (h d) -> p t h d", h=G))

        if do_proj:
            # K for pair 0 first (its matmuls start as x chunks stream in),
            # then the Q halves needed by the first two query blocks of
            # pair 0 so attention can start, then pair 1's K/Q.
            emit_qk_group(wk_sb, kT, 2, 0, 0)
            emit_qk_group(wk_sb, kT, 2, 0, 1)
            emit_qk_group(wq_sb, qT, 0, 0, 0)
            emit_qk_group(wk_sb, kT, 2, 1, 0)
            emit_qk_group(wk_sb, kT, 2, 1, 1)
            emit_qk_group(wq_sb, qT, 0, 1, 0)

        # ---- attention + output projection: per query block, head pairs
        # processed sequentially (pass p covers heads 2p, 2p+1). Block qb's
        # output projection is PACED into the following block's c-loops, one
        # m-chunk every other key chunk, so its matmuls and DVE evictions
        # stay in-stream (a fully deprioritized batch lets its evictions
        # park the in-order Vector queue and delay the next normalize).
        out_state = None
        pend_norm = None
        # (pass, c) -> outproj m-chunk firing slots
        OUT_SLOTS = {0: (5, 7, 9, 11, 13, 15), 1: (1, 5)}
        # pass -> Q^T half-slab to emit there (blocks 2/3, needed from qb2 on)
        HG_SCHED = {(0, 1): (0, 2), (1, 0): (0, 3), (1, 1): (1, 2),
                    (2, 0): (1, 3)}
        for qb in range(nqb if do_attn else 0):
            q0 = qb * QB
            last = (qb == nqb - 1)
            attn = []
            for p in range(2):
                av_ps = [ps_av.tile([HD + 1, QB], F32, tag="av", name="av_ps")
                         for _ in range(2)]
                for c in range(NKC):
                    if do_proj and qb == 0 and p == 0 and c % 2 == 0:
                        emit_v_2chunks(c // 2)  # V streams in ahead of its AV
                    if pend_norm is not None and c == 2:
                        pend_norm()
                        pend_norm = None
                    if (out_state is not None and c in OUT_SLOTS[p]
                            and out_state["m"] < NDC):
                        m = out_state["m"]
                        out_state["m"] += 1
                        oa, oq0 = out_state["attn"], out_state["q0"]
                        yp = ps_y.tile([P, QB], F32, tag="y", name="yp")
                        for h in range(2):
                            nc.tensor.matmul(
                                yp[:],
                                lhsT=wo_sb[:, h, m * P:(m + 1) * P],
                                rhs=oa[h][:],
                                start=(h == 0), stop=(h == 1))
                        ysb = smp.tile([P, QB], BF16, tag="ysb")
                        with nc.allow_low_precision(reason="bf16 partial y"):
                            nc.vector.tensor_copy(out=ysb, in_=yp[:])
                        nc.sync.dma_start(
                            out=yT[m * P:(m + 1) * P, oq0:oq0 + QB], in_=ysb)
                    if (do_proj and c == 8 and (qb, p) in HG_SCHED):
                        with tc.high_priority(offset=-1000000):
                            emit_q_halfgroup(*HG_SCHED[(qb, p)])
                    # last block: pair 0's output-projection half, one m-chunk
                    # per key chunk so it rides pair 1's PE slack
                    if last and p == 1 and do_outproj and 4 <= c < 4 + NDC:
                        m = c - 4
                        up = ps_y.tile([P, QB], F32, tag="y", name="up")
                        nc.tensor.matmul(
                            up[:],
                            lhsT=wo_sb[:, 0, m * P:(m + 1) * P],
                            rhs=attn[0][:],
                            start=True, stop=True)
                        nc.vector.tensor_copy(out=u_sb[:, m, :], in_=up[:])
                    c0 = c * P
                    s_ps = ps_s.tile([P, 2, QB], F32, tag="s")
                    for h2 in range(2):
                        base = HD * h2
                        nc.tensor.matmul(
                            s_ps[:, h2],
                            lhsT=kT[p][base:base + HD, c0:c0 + P],
                            rhs=qT[p][base:base + HD, q0:q0 + QB],
                            start=True, stop=True,
                            tile_position=(base, 0))
                    pt = ptp.tile([P, 2, QB], BF16, tag="pt")
                    with nc.allow_low_precision(reason="bf16 attn weights"):
                        nc.scalar.activation(out=pt[:], in_=s_ps[:],
                                             func=Exp if do_exp else
                                             mybir.ActivationFunctionType.Copy)
                    for h2 in range(2):
                        h = 2 * p + h2
                        nc.tensor.matmul(
                            av_ps[h2][:],
                            lhsT=v_sb[c // 2][:, c % 2, h, :],
                            rhs=pt[:, h2],
                            start=(c == 0), stop=(c == NKC - 1))

                # normalize stage 1 (immediate; DVE/DMA only): evict the
                # pair's attended^T + sums PSUM, 1/sums via fast reciprocal.
                # Stage 2 (the PE ones-broadcast + multiply + odd-head
                # relocation) is DEFERRED into the next pass's c-loop: every
                # matmul's completion feeds one global PE counter that later
                # consumers wait on, so a broadcast matmul parked on the
                # reciprocal chain at a pass boundary would stall the next
                # pass's scores — and with them the exp stream.
                at_pair = atp.tile([P, QB], BF16, tag="attn")
                av_sbs, rcs = [], []
                for h2 in range(2):
                    av_sb = smp.tile([HD + 1, QB], F32, tag="avsb")
                    nc.vector.tensor_copy(out=av_sb, in_=av_ps[h2][:])
                    av_sbs.append(av_sb)
                    if not do_norm:
                        continue
                    rr = tnp.tile([1, QB], F32, tag="rr")
                    nc.gpsimd.dma_start(out=rr[:, :], in_=av_sb[HD:HD + 1, :])
                    rcf = tnp.tile([1, QB], F32, tag="rcf")
                    nc.vector.reciprocal_approx_fast(out=rcf, in_=rr)
                    rc = tnp.tile([1, QB], BF16, tag="rcp")
                    with nc.allow_low_precision(reason="bf16 feed for PE bcast"):
                        nc.vector.tensor_copy(out=rc, in_=rcf)
                    rcs.append(rc)

                def pending_norm(at_pair=at_pair, av_sbs=av_sbs, rcs=rcs):
                    for h2 in range(2):
                        if not do_norm:
                            with nc.allow_low_precision(reason="bf16 attn"):
                                if h2 == 0:
                                    nc.vector.tensor_copy(
                                        out=at_pair[0:HD, :],
                                        in_=av_sbs[0][0:HD, :])
                                else:
                                    at_odd = smp.tile([HD, QB], BF16,
                                                      tag="atodd")
                                    nc.vector.tensor_copy(
                                        out=at_odd, in_=av_sbs[1][0:HD, :])
                                    nc.gpsimd.dma_start(out=at_pair[HD:P, :],
                                                        in_=at_odd[:, :])
                            continue
                        bc_ps = ps_y.tile([HD, QB], F32, tag="y", name="bc_ps")
                        nc.tensor.matmul(bc_ps[:], lhsT=ones_row,
                                         rhs=rcs[h2][:], start=True, stop=True)
                        with nc.allow_low_precision(reason="bf16 attn"):
                            if h2 == 0:
                                nc.vector.tensor_tensor(out=at_pair[0:HD, :],
                                                        in0=av_sbs[0][0:HD, :],
                                                        in1=bc_ps[:],
                                                        op=mybir.AluOpType.mult)
                            else:
                                at_odd = smp.tile([HD, QB], BF16, tag="atodd")
                                nc.vector.tensor_tensor(out=at_odd,
                                                        in0=av_sbs[1][0:HD, :],
                                                        in1=bc_ps[:],
                                                        op=mybir.AluOpType.mult)
                                nc.gpsimd.dma_start(out=at_pair[HD:P, :],
                                                    in_=at_odd[:, :])
                attn.append(at_pair)
                pend_norm = pending_norm

            if do_outproj and not last:
                out_state = {"attn": attn, "q0": q0, "m": 0}
            if last:
                final_attn, final_q0 = attn, q0

        if pend_norm is not None:
            pend_norm()
        if do_attn and do_outproj:
            # last block: u_sb already holds pair 0's half. Borrow the (now
            # idle) attended PSUM ring so the matmuls, adds, and DMAs of
            # successive m-chunks pipeline.
            for m in range(NDC):
                yp = ps_av.tile([P, QB], F32, tag="av", name="yp")
                nc.tensor.matmul(
                    yp[:],
                    lhsT=wo_sb[:, 1, m * P:(m + 1) * P],
                    rhs=final_attn[1][:],
                    start=True, stop=True)
                ysb = smp.tile([P, QB], BF16, tag="ysb")
                with nc.allow_low_precision(reason="bf16 partial y"):
                    nc.vector.tensor_tensor(out=ysb, in0=yp[:],
                                            in1=u_sb[:, m, :],
                                            op=mybir.AluOpType.add)
                nc.sync.dma_start(
                    out=yT[m * P:(m + 1) * P, final_q0:final_q0 + QB],
                    in_=ysb)

    nc.compile()
    return nc


_CACHE = {}


def _get_nc():
    if "nc" not in _CACHE:
        _CACHE["nc"] = _build_nc()
    return _CACHE["nc"]


def make_in_maps(x, Wq, bq, Wk, bk, Wv, bv, Wo, bo):
    """Host-side sharding: per-core input dicts for cores 0..7."""
    import ml_dtypes
    bf = ml_dtypes.bfloat16
    x = np.asarray(x, np.float32)
    scale = np.float32(1.0 / np.sqrt(HD))
    Wq_s = np.asarray(Wq, np.float32) * scale
    bq_s = np.asarray(bq, np.float32) * scale
    Wk = np.asarray(Wk, np.float32)
    bk = np.asarray(bk, np.float32)
    Wv = np.asarray(Wv, np.float32)
    Wo = np.asarray(Wo, np.float32)

    C = np.zeros((P, 768), bf)
    C[:, 0] = 1.0
    C[0, 641:641 + HD] = 1.0
    C[:, 740:748] = 1.0

    xts = [np.ascontiguousarray(x[b].T.astype(bf)) for b in range(2)]
    in_maps = []
    for core in range(8):
        b, g = divmod(core, 4)
        cols = slice(g * GD, (g + 1) * GD)
        bias = np.zeros((P, 4), np.float32)
        bias[:, 0] = bq_s[g * GD:g * GD + P]
        bias[:, 1] = bq_s[g * GD + P:(g + 1) * GD]
        bias[:, 2] = bk[g * GD:g * GD + P]
        bias[:, 3] = bk[g * GD + P:(g + 1) * GD]
        in_maps.append({
            "cst": C,
            "xT": xts[b],
            "wq": np.ascontiguousarray(Wq_s[:, cols].astype(bf)),
            "wk": np.ascontiguousarray(Wk[:, cols].astype(bf)),
            "wv": np.ascontiguousarray(Wv[:, cols].astype(bf)),
            "wo": np.ascontiguousarray(Wo[cols, :].astype(bf)),
            "bias": bias,
        })
    return in_maps


def gather_output(results, Wv, bv, Wo, bo):
    """Sum per-core partial y^T outputs and fold bv/bo exactly."""
    y = np.zeros((2, S, D), np.float32)
    for core in range(8):
        b = core // 4
        y[b] += results[core]["yT"].T.astype(np.float32)
    y += np.asarray(bo, np.float32) + np.asarray(bv, np.float32) @ np.asarray(Wo, np.float32)
    return y


def kernel(x, Wq, bq, Wk, bk, Wv, bv, Wo, bo):
    global LAST_RESULTS
    from concourse.bass_utils import run_bass_kernel_spmd
    in_maps = make_in_maps(x, Wq, bq, Wk, bk, Wv, bv, Wo, bo)
    res = run_bass_kernel_spmd(_get_nc(), in_maps, core_ids=list(range(8)),
                               trace=TRACE)
    LAST_RESULTS = res
    return gather_output(res.results, Wv, bv, Wo, bo)


# revision 35
# speedup vs baseline: 1.0858x; 1.0144x over previous
"""Trainium2 Bass kernel for a 16-head self-attention block.

Model (matches the nn.Module reference):
    q = x @ Wq + bq; k = x @ Wk + bk; v = x @ Wv + bv   (per-head split, Hd=64)
    attn = softmax(q k^T / sqrt(Hd)); out = (attn v) @ Wo + bo
Shapes: x [2, 2048, 1024], 16 heads, head dim 64.

Sharding (8 cores): core = (batch b in {0,1}) x (head-group g in {0..3});
each core owns 4 heads of one batch element. Inputs are sliced on the host;
each core returns a partial y^T = (attended_g @ Wo_g)^T which the host sums
over the 4 head-groups per batch.

Per-core design (bf16 matmul operands, fp32 PSUM accumulation — fp32r runs
fp32_mode=HIGH two-pass on hardware, ~2x the bf16 stream time):
  - Host passes xT = x[b]^T so projections need no on-device transpose.
  - Scores are computed transposed, S^T[key, q] = K_h Q_h^T, so softmax's
    exp runs straight out of PSUM on the Scalar engine and A = P V consumes
    P^T with no transpose anywhere. The two heads of a pair run as
    CONCURRENT row-group matmuls (K=64 at PE row offsets 0/64), so a score
    pair costs ~one 512-row pass.
  - softmax skips the max subtraction (mathematically identical; scores are
    O(5) here and ACT exp is <=2 ULP on [-10,10]).
  - P row sums come from a ones column appended to V (A' = P [V|1]), so the
    A.V matmul (M=65) yields attended^T plus the sums row with no extra
    matmuls or PSUM banks.
  - Normalization: sums row -> partition 0 via tiny SBUF->SBUF DMA, fast
    approx reciprocal (+cast to bf16), broadcast to 64 rows with a K=1 ones
    outer-product on the PE, then one vector multiply per head.
  - 1/sqrt(Hd) is folded into Wq (and bq) on the host; bv and bo are folded
    in exactly on the host: y += bo + bv @ Wo (softmax rows sum to 1).

Scheduling notes (exp stream is the critical resource, ~1.15us per key
chunk; everything else must fit underneath it):
  - PSUM banks: scores 2x[128,2,512] (4) + attended 3x[65,512] (3) +
    scratch ring 1x[128,512] (1) = 8. Three attended bufs let the next
    pair's first A.V start before the previous pair's PSUM eviction.
  - V is projected just-in-time during block 0 pair 0, two key chunks per
    PSUM tile.
  - The Q^T halves for blocks 2/3 are emitted during block 1 as
    single-bank groups in the scratch ring (not the score ring, which
    would stall the exp stream behind their deprioritized eviction).
  - Output projection of block qb is emitted a few chunks into block qb+1
    (PE slack). For the LAST block, pair 0's half runs during pair 1's
    c-loop into u_sb, and the tail only runs pair 1's matmuls + an add.
  - Input DMAs are split across the two hardware DGE queues (sync + ACT)
    and wk/wq are sliced per contraction chunk so the first projection
    matmuls start as soon as x chunk 0 lands.
"""

import numpy as np

import concourse.bass as bass
import concourse.tile as tile
from concourse import bacc
from concourse import mybir

P = 128          # partitions
S = 2048         # sequence length
D = 1024         # model dim
H = 16           # total heads
HD = 64          # head dim
G = 4            # heads per core
GD = G * HD      # 256 head-group dims per core
NQB = 4          # query blocks
QB = S // NQB    # 512
NKC = S // P     # 16 key chunks
NDC = D // P     # 8 contraction chunks
F32 = mybir.dt.float32
BF16 = mybir.dt.bfloat16

TRACE = False
LAST_RESULTS = None


def _build_nc(nqb=NQB, do_attn=True, do_exp=True, do_outproj=True,
              do_norm=True, do_proj=True, do_dma=True):
    nc = bacc.Bacc(trn_type="TRN2")
    xT = nc.dram_tensor("xT", [D, S], BF16, kind="ExternalInput")
    wq = nc.dram_tensor("wq", [D, GD], BF16, kind="ExternalInput")
    wk = nc.dram_tensor("wk", [D, GD], BF16, kind="ExternalInput")
    wv = nc.dram_tensor("wv", [D, GD], BF16, kind="ExternalInput")
    wo = nc.dram_tensor("wo", [GD, D], BF16, kind="ExternalInput")
    wo3 = nc.dram_tensor("wo3", [HD, D], BF16, kind="ExternalInput")
    bias = nc.dram_tensor("bias", [P, 4], F32, kind="ExternalInput")
    cst = nc.dram_tensor("cst", [P, 768], BF16, kind="ExternalInput")
    yT = nc.dram_tensor("yT", [D, S], BF16, kind="ExternalOutput")

    Exp = mybir.ActivationFunctionType.Exp
    Ident = mybir.ActivationFunctionType.Identity

    with tile.TileContext(nc) as tc, \
         tc.tile_pool(name="sb", bufs=1) as sb, \
         tc.tile_pool(name="pt", bufs=3) as ptp, \
         tc.tile_pool(name="small", bufs=2) as smp, \
         tc.tile_pool(name="attnp", bufs=5) as atp, \
         tc.tile_pool(name="tiny", bufs=2) as tnp, \
         tc.tile_pool(name="ps_s", bufs=2, space="PSUM") as ps_s, \
         tc.tile_pool(name="ps_av", bufs=3, space="PSUM") as ps_av, \
         tc.tile_pool(name="ps_y", bufs=1, space="PSUM") as ps_y:

        # ---- persistent SBUF tensors
        wq_sb = sb.tile([P, NDC, GD], BF16, tag="wq")
        wk_sb = sb.tile([P, NDC, GD], BF16, tag="wk")
        wv_sb = sb.tile([P, NDC, GD], BF16, tag="wv")
        wo_sb = sb.tile([P, 2, D], BF16, tag="wo")   # [pair-dims, pair, out-dim]
        wo3_sb = sb.tile([HD, D], BF16, tag="wo3")   # odd head of pair 1, at base 0
        bias_sb = sb.tile([P, 4], F32, tag="bias")
        scratch = sb.tile([P, 1], F32, tag="scratch")
        cst_sb = sb.tile([P, 768], BF16, tag="cst")
        ones_col = cst_sb[:, 0:1]             # [128, 1] ones
        ones_row = cst_sb[0:1, 641:641 + HD]  # [1, 64] ones
        ones8 = cst_sb[:, 740:748]            # [128, 8] ones
        u_sb = sb.tile([P, NDC, QB], F32, tag="u")   # last-block pair-0 y half
        x_sb = [sb.tile([P, S], BF16, tag=f"x{d}", name=f"x{d}") for d in range(NDC)]
        kT = [sb.tile([P, S], BF16, tag=f"k{p}", name=f"k{p}") for p in range(2)]
        qT = [sb.tile([P, S], BF16, tag=f"q{p}", name=f"q{p}") for p in range(2)]
        # V with a ones column per head, two key chunks per tile:
        # [keys, chunk-pair half, head, 65]
        v_sb = [sb.tile([P, 2, G, HD + 1], BF16, tag=f"v{j}", name=f"v{j}")
                for j in range(NKC // 2)]

        # ---- input DMAs (two hardware DGE queues: sync + scalar). Each wk
        # slice rides the same queue as its x chunk so a projection matmul
        # only ever waits on ONE queue semaphore; wq/wv/wo are covered by the
        # wtouch pre-observation below. Queues are balanced so x/wq/wv all
        # land as early as possible.
        if do_dma:
            nc.sync.dma_start(out=bias_sb, in_=bias[:, :])
            for d in range(NDC):
                eng = nc.sync if d % 2 == 0 else nc.scalar
                eng.dma_start(out=wk_sb[:, d, :], in_=wk[d * P:(d + 1) * P, :])
                eng.dma_start(out=x_sb[d], in_=xT[d * P:(d + 1) * P, :])
                if d == 1:
                    nc.scalar.dma_start(out=cst_sb, in_=cst[:, :])
            nc.sync.dma_start(out=wq_sb, in_=wq.rearrange("(o p) m -> p o m", p=P))
            nc.scalar.dma_start(out=wv_sb, in_=wv.rearrange("(o p) m -> p o m", p=P))
            nc.sync.dma_start(out=wo_sb, in_=wo.rearrange("(o p) m -> p o m", p=P))
            nc.scalar.dma_start(out=wo3_sb, in_=wo3[:, :])
        # warm the exp table set early so the ~2.7us load overlaps the prologue
        nc.scalar.activation(out=scratch, in_=ones_col, func=Exp)
        # V ones columns, written once up front (DVE is idle in the prologue)
        for j in range(NKC // 2):
            nc.vector.tensor_copy(
                out=v_sb[j][:, :, :, HD:HD + 1].rearrange("p a b c -> p (a b c)"),
                in_=ones8)

        # Pre-observe each weight DMA on the PE with a 1x1 dummy matmul, so
        # real matmuls never need two DMA-queue waits at once (walrus can't
        # encode >1 sync wait on an LDWEIGHTS).
        wtouch_ps = ps_y.tile([1, 4], F32, tag="y", name="wtouch")
        for i, w in enumerate((wk_sb, wv_sb, wq_sb)):
            nc.tensor.matmul(wtouch_ps[:, i:i + 1],
                             lhsT=w[0:1, 0, 0:1],
                             rhs=w[0:1, 0, 0:1],
                             start=True, stop=True)
        nc.tensor.matmul(wtouch_ps[:, 3:4],
                         lhsT=wo_sb[0:1, 0, 0:1],
                         rhs=wo_sb[0:1, 0, 0:1],
                         start=True, stop=True)

        # ---- projection emitters
        def emit_qk_group(w_sb, dst, bcol0, p, nb2):
            # one [128, 1024] output slab of K^T or Q^T; dst[p] [128, 2048]
            # rows 64*h2 hold head (2p+h2)'s 64 dims, columns are sequence.
            ps = ps_s.tile([P, 2, QB], F32, tag="s", name="qk_ps")
            for d in range(NDC):
                for half in range(2):
                    n0 = (2 * nb2 + half) * QB
                    nc.tensor.matmul(
                        ps[:, half],
                        lhsT=w_sb[:, d, p * P:(p + 1) * P],
                        rhs=x_sb[d][:, n0:n0 + QB],
                        start=(d == 0), stop=(d == NDC - 1))
            # evict with per-partition bias add
            with nc.allow_low_precision(reason="bf16 q/k for PE"):
                nc.scalar.activation(
                    out=dst[p][:, nb2 * 1024:(nb2 + 1) * 1024]
                        .rearrange("p (a b) -> p a b", a=2),
                    in_=ps[:],
                    func=Ident,
                    bias=bias_sb[:, bcol0 + p:bcol0 + p + 1],
                    scale=1.0)

        def emit_halfgroup(w_sb, dst, bcol, p, blk, ring):
            # one [128, 512] slab of K^T or Q^T for sequence block blk,
            # evicted on the Vector engine (per-partition bias add) so it
            # costs the exp stream nothing. ring="s": in-stream through the
            # score ring (prologue spill into block 0). ring="av": rides the
            # attended ring's third slot, lazy/deprioritized.
            if ring == "s":
                ps = ps_s.tile([P, QB], F32, tag="s", name="hg_ps")
            else:
                ps = ps_av.tile([P, QB], F32, tag="av", name="hg_ps")
            for d in range(NDC):
                nc.tensor.matmul(
                    ps[:],
                    lhsT=w_sb[:, d, p * P:(p + 1) * P],
                    rhs=x_sb[d][:, blk * QB:(blk + 1) * QB],
                    start=(d == 0), stop=(d == NDC - 1))
            with nc.allow_low_precision(reason="bf16 q/k for PE"):
                nc.vector.tensor_scalar_add(
                    out=dst[p][:, blk * QB:(blk + 1) * QB],
                    in0=ps[:],
                    scalar1=bias_sb[:, bcol + p:bcol + p + 1])

        def emit_v_2chunks(j):
            # v_sb[j] [128 keys, 2, head, 65] <- chunks 2j, 2j+1
            ps = ps_y.tile([P, 2, GD], F32, tag="y", name="v_ps")
            for t in range(2):
                c = 2 * j + t
                for d in range(NDC):
                    nc.tensor.matmul(
                        ps[:, t],
                        lhsT=x_sb[d][:, c * P:(c + 1) * P],
                        rhs=wv_sb[:, d, :],
                        start=(d == 0), stop=(d == NDC - 1))
            with nc.allow_low_precision(reason="bf16 v for PE"):
                nc.vector.tensor_copy(
                    out=v_sb[j][:, :, :, 0:HD],
                    in_=ps[:].rearrange("p t (h d) -> p t h d", h=G))

        if do_proj:
            # Only pair 0's K and first-half Q gate the first exp; pair 1's
            # K/Q stream through block 0 pass 0 as s-ring halfgroups (they
            # fill the PE bubbles of the V-projection chain there).
            emit_qk_group(wk_sb, kT, 2, 0, 0)
            emit_qk_group(wk_sb, kT, 2, 0, 1)
            emit_qk_group(wq_sb, qT, 0, 0, 0)

        # ---- attention + output projection: per query block, head pairs
        # processed sequentially (pass p covers heads 2p, 2p+1). Block qb's
        # output projection is PACED into the following block's c-loops, one
        # m-chunk every other key chunk, so its matmuls and DVE evictions
        # stay in-stream (a fully deprioritized batch lets its evictions
        # park the in-order Vector queue and delay the next normalize).
        out_state = None
        pend_norm = None
        final_odd = []
        # (pass, c) -> outproj m-chunk firing slots
        OUT_SLOTS = {0: (5, 7, 9, 11, 13, 15), 1: (1, 5)}
        # block-0 pass-0 c -> pair-1 K/Q halfgroup (s-ring, in-stream)
        HG_P1 = {1: (wk_sb, kT, 2, 0), 3: (wk_sb, kT, 2, 1),
                 5: (wk_sb, kT, 2, 2), 7: (wk_sb, kT, 2, 3),
                 9: (wq_sb, qT, 0, 0)}
        # pass -> Q^T half-slab for blocks 1-3 (av-ring, lazy)
        HG_SCHED = {(0, 1): (1, 1), (1, 0): (0, 2), (1, 1): (1, 2),
                    (2, 0): (0, 3), (2, 1): (1, 3)}
        for qb in range(nqb if do_attn else 0):
            q0 = qb * QB
            last = (qb == nqb - 1)
            attn = []
            for p in range(2):
                av_ps = [ps_av.tile([HD + 1, QB], F32, tag="av", name="av_ps")
                         for _ in range(2)]
                for c in range(NKC):
                    if do_proj and qb == 0 and p == 0 and c % 2 == 0:
                        emit_v_2chunks(c // 2)  # V streams in ahead of its AV
                    if pend_norm is not None and c == 2:
                        pend_norm()
                        pend_norm = None
                    if (out_state is not None and c in OUT_SLOTS[p]
                            and out_state["m"] < NDC):
                        m = out_state["m"]
                        out_state["m"] += 1
                        oa, oq0 = out_state["attn"], out_state["q0"]
                        yp = ps_y.tile([P, QB], F32, tag="y", name="yp")
                        for h in range(2):
                            nc.tensor.matmul(
                                yp[:],
                                lhsT=wo_sb[:, h, m * P:(m + 1) * P],
                                rhs=oa[h][:],
                                start=(h == 0), stop=(h == 1))
                        ysb = smp.tile([P, QB], BF16, tag="ysb")
                        with nc.allow_low_precision(reason="bf16 partial y"):
                            nc.vector.tensor_copy(out=ysb, in_=yp[:])
                        nc.sync.dma_start(
                            out=yT[m * P:(m + 1) * P, oq0:oq0 + QB], in_=ysb)
                    if do_proj and qb == 0 and p == 0 and c in HG_P1:
                        w, dst, bcol, blk = HG_P1[c]
                        emit_halfgroup(w, dst, bcol, 1, blk, ring="s")
                    if (do_proj and c == 8 and (qb, p) in HG_SCHED):
                        hp, hblk = HG_SCHED[(qb, p)]
                        with tc.high_priority(offset=-1000000):
                            emit_halfgroup(wq_sb, qT, 0, hp, hblk, ring="av")
                    # last block: pair 0's output-projection half, one m-chunk
                    # per key chunk so it rides pair 1's PE slack
                    if last and p == 1 and do_outproj and 4 <= c < 4 + NDC:
                        m = c - 4
                        up = ps_y.tile([P, QB], F32, tag="y", name="up")
                        nc.tensor.matmul(
                            up[:],
                            lhsT=wo_sb[:, 0, m * P:(m + 1) * P],
                            rhs=attn[0][:],
                            start=True, stop=True)
                        nc.vector.tensor_copy(out=u_sb[:, m, :], in_=up[:])
                    c0 = c * P
                    s_ps = ps_s.tile([P, 2, QB], F32, tag="s")
                    for h2 in range(2):
                        base = HD * h2
                        nc.tensor.matmul(
                            s_ps[:, h2],
                            lhsT=kT[p][base:base + HD, c0:c0 + P],
                            rhs=qT[p][base:base + HD, q0:q0 + QB],
                            start=True, stop=True,
                            tile_position=(base, 0))
                    pt = ptp.tile([P, 2, QB], BF16, tag="pt")
                    with nc.allow_low_precision(reason="bf16 attn weights"):
                        nc.scalar.activation(out=pt[:], in_=s_ps[:],
                                             func=Exp if do_exp else
                                             mybir.ActivationFunctionType.Copy)
                    for h2 in range(2):
                        h = 2 * p + h2
                        nc.tensor.matmul(
                            av_ps[h2][:],
                            lhsT=v_sb[c // 2][:, c % 2, h, :],
                            rhs=pt[:, h2],
                            start=(c == 0), stop=(c == NKC - 1))

                # normalize stage 1 (immediate; DVE/DMA only): evict the
                # pair's attended^T + sums PSUM, 1/sums via fast reciprocal.
                # Stage 2 (the PE ones-broadcast + multiply + odd-head
                # relocation) is DEFERRED into the next pass's c-loop: every
                # matmul's completion feeds one global PE counter that later
                # consumers wait on, so a broadcast matmul parked on the
                # reciprocal chain at a pass boundary would stall the next
                # pass's scores — and with them the exp stream.
                at_pair = atp.tile([P, QB], BF16, tag="attn")
                av_sbs, rcs = [], []
                for h2 in range(2):
                    av_sb = smp.tile([HD + 1, QB], F32, tag="avsb")
                    nc.vector.tensor_copy(out=av_sb, in_=av_ps[h2][:])
                    av_sbs.append(av_sb)
                    if not do_norm:
                        continue
                    rr = tnp.tile([1, QB], F32, tag="rr")
                    nc.gpsimd.dma_start(out=rr[:, :], in_=av_sb[HD:HD + 1, :])
                    rcf = tnp.tile([1, QB], F32, tag="rcf")
                    nc.vector.reciprocal_approx_fast(out=rcf, in_=rr)
                    rc = tnp.tile([1, QB], BF16, tag="rcp")
                    with nc.allow_low_precision(reason="bf16 feed for PE bcast"):
                        nc.vector.tensor_copy(out=rc, in_=rcf)
                    rcs.append(rc)

                keep_odd = last and p == 1

                def pending_norm(at_pair=at_pair, av_sbs=av_sbs, rcs=rcs,
                                 keep_odd=keep_odd):
                    for h2 in range(2):
                        if not do_norm:
                            with nc.allow_low_precision(reason="bf16 attn"):
                                if h2 == 0:
                                    nc.vector.tensor_copy(
                                        out=at_pair[0:HD, :],
                                        in_=av_sbs[0][0:HD, :])
                                else:
                                    at_odd = smp.tile([HD, QB], BF16,
                                                      tag="atodd")
                                    nc.vector.tensor_copy(
                                        out=at_odd, in_=av_sbs[1][0:HD, :])
                                    nc.gpsimd.dma_start(out=at_pair[HD:P, :],
                                                        in_=at_odd[:, :])
                            continue
                        bc_ps = ps_y.tile([HD, QB], F32, tag="y", name="bc_ps")
                        nc.tensor.matmul(bc_ps[:], lhsT=ones_row,
                                         rhs=rcs[h2][:], start=True, stop=True)
                        with nc.allow_low_precision(reason="bf16 attn"):
                            if h2 == 0:
                                nc.vector.tensor_tensor(out=at_pair[0:HD, :],
                                                        in0=av_sbs[0][0:HD, :],
                                                        in1=bc_ps[:],
                                                        op=mybir.AluOpType.mult)
                            else:
                                at_odd = smp.tile([HD, QB], BF16, tag="atodd")
                                nc.vector.tensor_tensor(out=at_odd,
                                                        in0=av_sbs[1][0:HD, :],
                                                        in1=bc_ps[:],
                                                        op=mybir.AluOpType.mult)
                                if keep_odd:
                                    # last pass: the final output projection
                                    # reads at_odd directly (split matmul),
                                    # skipping the relocation DMA latency
                                    final_odd.append(at_odd)
                                else:
                                    nc.gpsimd.dma_start(out=at_pair[HD:P, :],
                                                        in_=at_odd[:, :])
                attn.append(at_pair)
                pend_norm = pending_norm

            if do_outproj and not last:
                out_state = {"attn": attn, "q0": q0, "m": 0}
            if last:
                final_attn, final_q0 = attn, q0

        if pend_norm is not None:
            pend_norm()
        if do_attn and do_outproj:
            # last block: u_sb already holds pair 0's half. Split each
            # m-chunk into two K=64 matmuls so the second half reads at_odd
            # directly (no relocation DMA on the critical tail), pipeline
            # through the now-idle attended PSUM ring, and ship the final y
            # chunks on the scalar DGE queue (the sync queue still drains
            # earlier output blocks).
            for m in range(NDC):
                yp = ps_av.tile([P, QB], F32, tag="av", name="yp")
                nc.tensor.matmul(
                    yp[:],
                    lhsT=wo_sb[0:HD, 1, m * P:(m + 1) * P],
                    rhs=final_attn[1][0:HD, :],
                    start=True, stop=False)
                nc.tensor.matmul(
                    yp[:],
                    lhsT=wo3_sb[:, m * P:(m + 1) * P],
                    rhs=final_odd[0][:, :],
                    start=False, stop=True)
                ysb = smp.tile([P, QB], BF16, tag="ysb")
                with nc.allow_low_precision(reason="bf16 partial y"):
                    nc.vector.tensor_tensor(out=ysb, in0=yp[:],
                                            in1=u_sb[:, m, :],
                                            op=mybir.AluOpType.add)
                nc.scalar.dma_start(
                    out=yT[m * P:(m + 1) * P, final_q0:final_q0 + QB],
                    in_=ysb)

    nc.compile()
    return nc


_CACHE = {}


def _get_nc():
    if "nc" not in _CACHE:
        _CACHE["nc"] = _build_nc()
    return _CACHE["nc"]


def make_in_maps(x, Wq, bq, Wk, bk, Wv, bv, Wo, bo):
    """Host-side sharding: per-core input dicts for cores 0..7."""
    import ml_dtypes
    bf = ml_dtypes.bfloat16
    x = np.asarray(x, np.float32)
    scale = np.float32(1.0 / np.sqrt(HD))
    Wq_s = np.asarray(Wq, np.float32) * scale
    bq_s = np.asarray(bq, np.float32) * scale
    Wk = np.asarray(Wk, np.float32)
    bk = np.asarray(bk, np.float32)
    Wv = np.asarray(Wv, np.float32)
    Wo = np.asarray(Wo, np.float32)

    C = np.zeros((P, 768), bf)
    C[:, 0] = 1.0
    C[0, 641:641 + HD] = 1.0
    C[:, 740:748] = 1.0

    xts = [np.ascontiguousarray(x[b].T.astype(bf)) for b in range(2)]
    in_maps = []
    for core in range(8):
        b, g = divmod(core, 4)
        cols = slice(g * GD, (g + 1) * GD)
        bias = np.zeros((P, 4), np.float32)
        bias[:, 0] = bq_s[g * GD:g * GD + P]
        bias[:, 1] = bq_s[g * GD + P:(g + 1) * GD]
        bias[:, 2] = bk[g * GD:g * GD + P]
        bias[:, 3] = bk[g * GD + P:(g + 1) * GD]
        in_maps.append({
            "cst": C,
            "xT": xts[b],
            "wq": np.ascontiguousarray(Wq_s[:, cols].astype(bf)),
            "wk": np.ascontiguousarray(Wk[:, cols].astype(bf)),
            "wv": np.ascontiguousarray(Wv[:, cols].astype(bf)),
            "wo": np.ascontiguousarray(Wo[cols, :].astype(bf)),
            "wo3": np.ascontiguousarray(Wo[cols, :][3 * HD:4 * HD, :].astype(bf)),
            "bias": bias,
        })
    return in_maps


def gather_output(results, Wv, bv, Wo, bo):
    """Sum per-core partial y^T outputs and fold bv/bo exactly."""
    y = np.zeros((2, S, D), np.float32)
    for core in range(8):
        b = core // 4
        y[b] += results[core]["yT"].T.astype(np.float32)
    y += np.asarray(bo, np.float32) + np.asarray(bv, np.float32) @ np.asarray(Wo, np.float32)
    return y


def kernel(x, Wq, bq, Wk, bk, Wv, bv, Wo, bo):
    global LAST_RESULTS
    from concourse.bass_utils import run_bass_kernel_spmd
    in_maps = make_in_maps(x, Wq, bq, Wk, bk, Wv, bv, Wo, bo)
    res = run_bass_kernel_spmd(_get_nc(), in_maps, core_ids=list(range(8)),
                               trace=TRACE)
    LAST_RESULTS = res
    return gather_output(res.results, Wv, bv, Wo, bo)
